# revision 19
# baseline (speedup 1.0000x reference)
"""Trainium2 Bass kernel for nn_Aggregation_89575837925422.

Module: feats = [x, dconv3(x), dconv5(x), dconv7(x)] (1920 ch); qk = w_qk@feats;
4-head attention with relative-position biases; out = x + gamma*proj(attn@v).

Sharding across 8 NeuronCores: core c = (batch b=c//4, row-slab s=c%4, 11 rows).
Each core: all conv channels for its slab -> k(all heads)+q at slab positions ->
AllGather of k within the batch group -> v^T for its slab positions, AllGathered
across the group (each core computes 1/4 of v^T) -> all 4 heads' attention for
its query slab (augmented-K matmul folds rel-pos biases into the logit matmul;
softmax without max-subtraction via constant shift; denominators via a ones
column in the v operand) -> proj + residual. Output per core: (768, 484).

All matmul operands are bf16 (f32 PSUM accumulation): full PE rate, half the
HBM/SBUF traffic of f32r, keeping the tensor engine fed so its DVFS clock stays
in the high p-states. f32 is kept for the residual x, softmax denominators, and
the final output.
"""
import os
import sys

sys.path.insert(0, "/opt/trn_rl_repo")

from contextlib import ExitStack

import ml_dtypes
import numpy as np

import concourse.bacc as bacc
import concourse.mybir as mybir
import concourse.tile as tile
from concourse.bass_utils import run_bass_kernel_spmd

dt = mybir.dt
F32 = dt.float32
BF16 = dt.bfloat16
AF = mybir.ActivationFunctionType
BF = ml_dtypes.bfloat16

# ---- problem constants (hardcoded; kernel.py must be self-contained) ----
B = 2
CIN = 768
S = 44
HW = S * S              # 1936
HEAD = 4
D = 192                 # head dim
CO = 384                # conv out channels per dilation
DILS = (3, 5, 7)
NSLAB = 4
SLABR = 11              # rows per slab
NPOS = SLABR * S        # 484
MAXPOS = 100
KC_X = CIN // 128       # 6
C_CONV = 3 * CO         # 1152
MB_CONV = C_CONV // 128  # 9
KC_F = KC_X + MB_CONV   # 15
PADR = 7 + SLABR + 7    # 25
PADW = 64               # 7+44+13 zero pad, 128B row pitch in bf16
NJC = 16                # j-chunks of 121 rows each (16*121 = 1936)
JCH = HW // NJC         # 121
VT_W = HEAD * (D + 1)   # 772
EXP_SHIFT = -20.0       # exp(sim - 20); cancels in softmax, avoids fp32 overflow
NCORES = 8
GROUPS = [[0, 1, 2, 3], [4, 5, 6, 7]]


# ---------------------------------------------------------------------------
# host-side input preparation
# ---------------------------------------------------------------------------

def prep_shared(inputs):
    out = {}
    w_conv = np.empty((MB_CONV, 128, 54 * 128), np.float32)
    b_conv = np.empty((128, MB_CONV), np.float32)
    for mb in range(MB_CONV):
        dil_i, mloc = mb // 3, mb % 3
        Wd = np.asarray(inputs[f"w_d{DILS[dil_i]}"], np.float32)
        Wb = Wd[mloc * 128:(mloc + 1) * 128]              # (128m, 768ci, 3, 3)
        t = Wb.reshape(128, KC_X, 128, 3, 3).transpose(3, 4, 1, 2, 0)
        w_conv[mb] = t.reshape(54, 128, 128).transpose(1, 0, 2).reshape(128, 54 * 128)
        b_conv[:, mb] = np.asarray(inputs[f"b_d{DILS[dil_i]}"], np.float32)[mloc * 128:(mloc + 1) * 128]
    out["w_conv"] = w_conv.astype(BF)
    out["b_conv"] = b_conv

    w_qk = np.asarray(inputs["w_qk"], np.float32)         # (1536, 1920)
    qscale = HEAD ** -0.5
    w_qk_l = np.empty((12, 128, KC_F * 128), np.float32)
    for blk in range(12):
        rows = w_qk[blk * 128:(blk + 1) * 128]
        scale = qscale if blk < 6 else 1.0
        w_qk_l[blk] = (rows * scale).reshape(128, KC_F, 128).transpose(2, 1, 0).reshape(
            128, KC_F * 128)
    out["w_qk"] = w_qk_l.astype(BF)

    w_v = np.asarray(inputs["w_v"], np.float32)           # (768, 768) [o, c]
    w_vt = np.zeros((KC_X, 128, VT_W), np.float32)
    for kc in range(KC_X):
        blockT = w_v[:, kc * 128:(kc + 1) * 128].T
        for h in range(HEAD):
            w_vt[kc][:, h * (D + 1):h * (D + 1) + D] = blockT[:, h * D:(h + 1) * D]
    out["w_vt"] = w_vt.astype(BF)

    gamma = float(np.asarray(inputs["gamma"]).reshape(-1)[0])
    w_proj = np.asarray(inputs["w_proj"], np.float32)
    wpa = np.empty((HEAD, 128, CIN), np.float32)
    wpb = np.empty((HEAD, 64, CIN), np.float32)
    for h in range(HEAD):
        wpa[h] = gamma * w_proj[:, h * D:h * D + 128].T
        wpb[h] = gamma * w_proj[:, h * D + 128:(h + 1) * D].T
    out["w_proj_a"] = wpa.astype(BF)
    out["w_proj_b"] = wpb.astype(BF)

    rel_w = np.asarray(inputs["rel_w"], np.float32)
    iy = np.arange(S)
    rw = rel_w[iy[None, :] - iy[:, None] + MAXPOS - 1]    # (y, v, d)
    out["rw"] = np.ascontiguousarray(rw.transpose(2, 0, 1).reshape(D, S * S)).astype(BF)

    j = np.arange(HW)
    U = (j[None, :] // S == np.arange(S)[:, None]).astype(np.float32)
    V = (j[None, :] % S == np.arange(S)[:, None]).astype(np.float32)
    out["k2c"] = np.ascontiguousarray(np.concatenate([U, V[:20]], axis=0)).astype(BF)  # (64, 1936)
    out["k3c"] = np.ascontiguousarray(V[20:44]).astype(BF)                             # (24, 1936)
    return out


def prep_core(inputs, core):
    b, s = core // 4, core % 4
    r0 = s * SLABR
    x = np.asarray(inputs["x"], np.float32)[b]
    out = {}
    xp = np.zeros((KC_X, 128, PADR, PADW), np.float32)
    rlo, rhi = r0 - 7, r0 + SLABR + 7
    glo, ghi = max(rlo, 0), min(rhi, S)
    xr = x.reshape(KC_X, 128, S, S)
    xp[:, :, glo - rlo:ghi - rlo, 7:7 + S] = xr[:, :, glo:ghi, :]
    out["x_pad"] = xp.reshape(KC_X, 128, PADR * PADW).astype(BF)
    xf = xr.reshape(KC_X, 128, HW)
    out["x_full"] = np.ascontiguousarray(xf).astype(BF)
    xs = np.ascontiguousarray(xf[:, :, r0 * S:r0 * S + NPOS])
    out["x_slab_bf"] = xs.astype(BF)
    out["x_res"] = xs
    rel_h = np.asarray(inputs["rel_h"], np.float32)
    ix = np.arange(S)
    rh = rel_h[ix[None, :] - ix[:, None] + MAXPOS - 1]    # (x, u, d)
    out["rh"] = np.ascontiguousarray(
        rh[r0:r0 + SLABR].transpose(2, 0, 1).reshape(D, NPOS)).astype(BF)
    return out


def make_in_maps(inputs):
    shared = prep_shared(inputs)
    in_maps = []
    for c in range(NCORES):
        m = dict(shared)
        m.update(prep_core(inputs, c))
        in_maps.append(m)
    return in_maps


# ---------------------------------------------------------------------------
# Bass program (SPMD, identical on all 8 cores)
# ---------------------------------------------------------------------------

def build_nc():
    nc = bacc.Bacc("TRN2", target_bir_lowering=False, debug=False,
                   num_devices=NCORES)

    d_in = {}
    for name, shape in [
        ("x_pad", [KC_X, 128, PADR * PADW]),
        ("x_slab_bf", [KC_X, 128, NPOS]),
        ("x_full", [KC_X, 128, HW]),
        ("w_conv", [MB_CONV, 128, 54 * 128]),
        ("w_qk", [12, 128, KC_F * 128]),
        ("w_vt", [KC_X, 128, VT_W]),
        ("w_proj_a", [HEAD, 128, CIN]),
        ("w_proj_b", [HEAD, 64, CIN]),
        ("rh", [D, NPOS]),
        ("rw", [D, HW]),
        ("k2c", [64, HW]),
        ("k3c", [24, HW]),
    ]:
        d_in[name] = nc.dram_tensor(name, shape, BF16, kind="ExternalInput")
    d_in["b_conv"] = nc.dram_tensor("b_conv", [128, MB_CONV], F32, kind="ExternalInput")
    d_in["x_res"] = nc.dram_tensor("x_res", [KC_X, 128, NPOS], F32, kind="ExternalInput")
    out_d = nc.dram_tensor("out", [CIN, NPOS], F32, kind="ExternalOutput")

    with tile.TileContext(nc) as tc:
        with ExitStack() as ctx:
            # persistent pools (whole kernel)
            consts = ctx.enter_context(tc.tile_pool(name="consts", bufs=1))
            work = ctx.enter_context(tc.tile_pool(name="work", bufs=2))
            vtpool = ctx.enter_context(tc.tile_pool(name="vtpool", bufs=1))
            qpool = ctx.enter_context(tc.tile_pool(name="qpool", bufs=1))
            ps2 = ctx.enter_context(tc.tile_pool(name="ps2", bufs=2, space="PSUM"))
            dram = ctx.enter_context(tc.tile_pool(name="dram", bufs=1, space="DRAM"))

            # phase-scoped pools (closed manually, strict LIFO order)
            sXF = ExitStack()  # x_full for the local v^T compute
            xfpool = sXF.enter_context(tc.tile_pool(name="xfpool", bufs=1))
            sB = ExitStack()   # conv outputs + qk weights (closes after qk)
            convout = sB.enter_context(tc.tile_pool(name="convout", bufs=1))
            qkw = sB.enter_context(tc.tile_pool(name="qkw", bufs=2))
            sA = ExitStack()   # conv inputs/weights (closes after conv)
            xpool = sA.enter_context(tc.tile_pool(name="xpool", bufs=1))
            convw = sA.enter_context(tc.tile_pool(name="convw", bufs=8))

            # ---- persistent input tiles ----
            # x_pad first on the sync queue: the first conv matmul needs it
            xpad_t = []
            for kc in range(KC_X):
                t = xpool.tile([128, PADR * PADW], BF16, tag=f"xpad{kc}")
                nc.sync.dma_start(t[:], d_in["x_pad"].ap()[kc])
                xpad_t.append(t)
            xslab_t = []
            for kc in range(KC_X):
                t = consts.tile([128, NPOS], BF16, tag=f"xslab{kc}")
                nc.scalar.dma_start(t[:], d_in["x_slab_bf"].ap()[kc])
                xslab_t.append(t)
            bconv_t = consts.tile([128, MB_CONV], F32, tag="bconv")
            nc.scalar.dma_start(bconv_t[:], d_in["b_conv"].ap())
            # rel-pos tables: early, on the scalar queue (gpsimd's queue stalls
            # behind collective triggers)
            rh1 = consts.tile([128, NPOS], BF16, tag="rh1")
            nc.scalar.dma_start(rh1[:], d_in["rh"].ap()[0:128, :])
            rh2 = consts.tile([64, NPOS], BF16, tag="rh2")
            nc.scalar.dma_start(rh2[:], d_in["rh"].ap()[128:D, :])
            rw1 = consts.tile([128, HW], BF16, tag="rw1")
            nc.scalar.dma_start(rw1[:], d_in["rw"].ap()[0:128, :])
            rw2 = consts.tile([64, HW], BF16, tag="rw2")
            nc.scalar.dma_start(rw2[:], d_in["rw"].ap()[128:D, :])
            wvt_t = []
            xfull_t = []
            for kc in range(KC_X):
                t = consts.tile([128, VT_W], BF16, tag=f"wvt{kc}")
                nc.gpsimd.dma_start(t[:], d_in["w_vt"].ap()[kc])
                wvt_t.append(t)
                t = xfpool.tile([128, HW], BF16, tag=f"xfull{kc}")
                nc.gpsimd.dma_start(t[:], d_in["x_full"].ap()[kc])
                xfull_t.append(t)

            # ---- DRAM scratch ----
            ag_in_a = dram.tile([CIN // 2, NPOS], BF16, tag="ag_in_a")
            ag_in_b = dram.tile([CIN // 2, NPOS], BF16, tag="ag_in_b")
            ag_out_a = dram.tile([NSLAB * CIN // 2, NPOS], BF16, tag="ag_out_a")
            ag_out_b = dram.tile([NSLAB * CIN // 2, NPOS], BF16, tag="ag_out_b")

            # ---- stage 1: dilated convs for this slab ----
            sid_conv = nc.enter_named_scope("conv", False)[0]
            conv_out = []
            for mb in range(MB_CONV):
                cps = ps2.tile([128, NPOS], F32, tag="mm484")
                first = True
                for kh in range(3):
                    for kw in range(3):
                        t_idx = kh * 3 + kw
                        wt = convw.tile([128, KC_X * 128], BF16, tag="convw")
                        nc.sync.dma_start(
                            wt[:],
                            d_in["w_conv"].ap()[mb][:, t_idx * KC_X * 128:(t_idx + 1) * KC_X * 128],
                        )
                        dil = DILS[mb // 3]
                        dr, dw = dil * (kh - 1), dil * (kw - 1)
                        for kc in range(KC_X):
                            rhs = xpad_t[kc][:].rearrange(
                                "p (r w) -> p r w", w=PADW
                            )[:, 7 + dr:7 + dr + SLABR, 7 + dw:7 + dw + S]
                            last = (t_idx == 8 and kc == KC_X - 1)
                            nc.tensor.matmul(
                                cps[:], wt[:, kc * 128:(kc + 1) * 128], rhs,
                                start=first, stop=last,
                            )
                            first = False
                co = convout.tile([128, NPOS], BF16, tag=f"conv{mb}")
                nc.scalar.activation(co[:], cps[:], AF.Identity,
                                     bias=bconv_t[:, mb:mb + 1])
                conv_out.append(co)
            sA.close()
            nc.leave_named_scope("conv", sid_conv, False)

            def feats_rhs(kc):
                if kc < KC_X:
                    return xslab_t[kc][:]
                return conv_out[kc - KC_X][:]

            # q lives in SBUF for the whole kernel: 6 blocks of 128 rows
            q_sb = [qpool.tile([128, NPOS], BF16, tag=f"qsb{b}", name=f"qsb{b}")
                    for b in range(KC_X)]

            def q_row_splits(r0g, n):
                """Split q global rows [r0g, r0g+n) into (tile, src_off, dst_off, n)."""
                parts = []
                done = 0
                while done < n:
                    t_i, off = divmod(r0g + done, 128)
                    take = min(128 - off, n - done)
                    parts.append((t_i, off, done, take))
                    done += take
                return parts

            # ---- stage 2a: k blocks + AllGathers ----
            sid_qk = nc.enter_named_scope("qk_k", False)[0]

            def qk_block(blk, dst, dst_row):
                qps = ps2.tile([128, NPOS], F32, tag="mm484", name=f"qps{blk}")
                wt = qkw.tile([128, KC_F * 128], BF16, tag="qkw", name=f"qkw{blk}")
                nc.scalar.dma_start(wt[:], d_in["w_qk"].ap()[blk])
                for kc in range(KC_F):
                    nc.tensor.matmul(qps[:], wt[:, kc * 128:(kc + 1) * 128],
                                     feats_rhs(kc),
                                     start=(kc == 0), stop=(kc == KC_F - 1))
                if dst is None:
                    nc.vector.tensor_copy(q_sb[blk][:], qps[:])
                else:
                    sb = qkw.tile([128, NPOS], BF16, tag="qkout", name=f"qko{blk}")
                    nc.vector.tensor_copy(sb[:], qps[:])
                    nc.sync.dma_start(dst[dst_row:dst_row + 128, :], sb[:])

            for blk in range(6, 9):
                qk_block(blk, ag_in_a, (blk - 6) * 128)
            sid_ag = nc.enter_named_scope("allgather", False)[0]
            nc.gpsimd.collective_compute(
                "AllGather", mybir.AluOpType.bypass,
                ins=[ag_in_a[:]], outs=[ag_out_a[:]], replica_groups=GROUPS,
            )
            nc.leave_named_scope("allgather", sid_ag, False)
            for blk in range(9, 12):
                qk_block(blk, ag_in_b, (blk - 9) * 128)
            sid_ag2 = nc.enter_named_scope("allgather2", False)[0]
            nc.gpsimd.collective_compute(
                "AllGather", mybir.AluOpType.bypass,
                ins=[ag_in_b[:]], outs=[ag_out_b[:]], replica_groups=GROUPS,
            )
            nc.leave_named_scope("allgather2", sid_ag2, False)
            nc.leave_named_scope("qk_k", sid_qk, False)

            # ---- stage 3: vT_wide = x^T @ w_vT (all positions, local) ----
            sid_vt = nc.enter_named_scope("vt", False)[0]
            sC = ExitStack()
            psC = sC.enter_context(tc.tile_pool(name="psC", bufs=2, space="PSUM"))
            vt_t = []
            for jc in range(NJC):
                j0 = jc * JCH
                vps = psC.tile([128, VT_W], F32, tag="vtps")
                for kc in range(KC_X):
                    lhsT = xfull_t[kc][:, j0:j0 + JCH]
                    nc.tensor.matmul(vps[:JCH, 0:512], lhsT, wvt_t[kc][:, 0:512],
                                     start=(kc == 0), stop=(kc == KC_X - 1))
                    nc.tensor.matmul(vps[:JCH, 512:VT_W], lhsT,
                                     wvt_t[kc][:, 512:VT_W],
                                     start=(kc == 0), stop=(kc == KC_X - 1))
                t = vtpool.tile([128, VT_W], BF16, tag=f"vt{jc}", name=f"vt{jc}")
                nc.vector.tensor_copy(t[:JCH], vps[:JCH])
                for h in range(HEAD):
                    col = h * (D + 1) + D
                    nc.vector.memset(t[:JCH, col:col + 1], 1.0)
                vt_t.append(t)
            sC.close()
            nc.leave_named_scope("vt", sid_vt, False)

            # ---- stage 2b: q blocks (into SBUF) ----
            sid_qq = nc.enter_named_scope("qk_q", False)[0]
            for blk in range(6):
                qk_block(blk, None, 0)
            nc.leave_named_scope("qk_q", sid_qq, False)
            sB.close()
            sXF.close()

            # proj weights + attn output tiles (persist to the end)
            sPJ = ExitStack()
            projpool = sPJ.enter_context(tc.tile_pool(name="projpool", bufs=1))
            wpa_t, wpb_t = [], []
            for h in range(HEAD):
                ta = projpool.tile([128, CIN], BF16, tag=f"wpa{h}", name=f"wpa{h}")
                nc.sync.dma_start(ta[:], d_in["w_proj_a"].ap()[h])
                wpa_t.append(ta)
                tb = projpool.tile([64, CIN], BF16, tag=f"wpb{h}", name=f"wpb{h}")
                nc.sync.dma_start(tb[:], d_in["w_proj_b"].ap()[h])
                wpb_t.append(tb)
            # attention outputs, written in place by the normalize multiply
            at_a = [projpool.tile([128, NPOS], BF16, tag=f"ata{h}", name=f"ata{h}")
                    for h in range(HEAD)]
            at_b = [projpool.tile([64, NPOS], BF16, tag=f"atb{h}", name=f"atb{h}")
                    for h in range(HEAD)]
            # residual x (f32) for the final add; loaded late on gpsimd queue
            xres_t = []
            for kc in range(KC_X):
                t = projpool.tile([128, NPOS], F32, tag=f"xres{kc}", name=f"xres{kc}")
                nc.gpsimd.dma_start(t[:], d_in["x_res"].ap()[kc])
                xres_t.append(t)
            # f32 partial proj accumulators (pair0's half of the contraction)
            pjpart_t = [projpool.tile([128, NPOS], F32, tag=f"pjp{mb}", name=f"pjp{mb}")
                        for mb in range(KC_X)]

            # rel-pos tables + attention staging pools
            relout = ExitStack()
            relout_pool = relout.enter_context(tc.tile_pool(name="relout", bufs=1))
            sE = ExitStack()
            augpool = sE.enter_context(tc.tile_pool(name="augpool", bufs=4))
            aug1pool = sE.enter_context(tc.tile_pool(name="aug1pool", bufs=1))
            exppool = sE.enter_context(tc.tile_pool(name="exppool", bufs=5))
            psE = sE.enter_context(tc.tile_pool(name="psE", bufs=1, space="PSUM"))
            psS = sE.enter_context(tc.tile_pool(name="psS", bufs=2, space="PSUM"))
            shift_t = consts.tile([128, 1], F32, tag="shift")
            nc.vector.memset(shift_t[:], EXP_SHIFT)
            ka3 = aug1pool.tile([24, HW], BF16, tag="ka3")
            nc.sync.dma_start(ka3[:], d_in["k3c"].ap())

            sD = ExitStack()
            qhpool = sD.enter_context(tc.tile_pool(name="qhpool", bufs=1))

            rht_sb = {}
            rwt_sb = {}

            def emit_rel():
                sid = nc.enter_named_scope("rel", False)[0]
                # q gathered once, x-major layout: col = a*176 + g*44 + b
                qhx1 = qhpool.tile([128, SLABR * 176], BF16, tag="qhx1", name="qhx1")
                qhx2 = qhpool.tile([64, SLABR * 176], BF16, tag="qhx2", name="qhx2")
                vx1 = qhx1[:].rearrange("p (a g b) -> p a g b", g=HEAD, b=S)
                vx2 = qhx2[:].rearrange("p (a g b) -> p a g b", g=HEAD, b=S)
                # same data viewed (g, a) per fixed y for the rwt matmuls
                vg1 = qhx1[:].rearrange("p (a g b) -> p g a b", g=HEAD, b=S)
                vg2 = qhx2[:].rearrange("p (a g b) -> p g a b", g=HEAD, b=S)
                for g in range(HEAD):
                    for (t_i, off, d0, take) in q_row_splits(g * D, 128):
                        src = q_sb[t_i][off:off + take, :]
                        nc.scalar.dma_start(vx1[d0:d0 + take, :, g, :], src)
                    for (t_i, off, d0, take) in q_row_splits(g * D + 128, 64):
                        src = q_sb[t_i][off:off + take, :]
                        nc.scalar.dma_start(vx2[d0:d0 + take, :, g, :], src)

                # rht[u, (x, g, y)]: one matmul pair per slab row x
                rht_all = qhpool.tile([S, SLABR * 176], BF16, tag="rht_all",
                                       name="rhta")
                for xl in range(SLABR):
                    rps = ps2.tile([S, 176], F32, tag="mm484", name=f"rhtp{xl}")
                    cs = slice(xl * S, (xl + 1) * S)
                    nc.tensor.matmul(rps[0:S, :], rh1[:, cs],
                                     qhx1[:, xl * 176:(xl + 1) * 176],
                                     start=True, stop=False)
                    nc.tensor.matmul(rps[0:S, :], rh2[:, cs],
                                     qhx2[:, xl * 176:(xl + 1) * 176],
                                     start=False, stop=True)
                    nc.vector.tensor_copy(rht_all[:, xl * 176:(xl + 1) * 176],
                                          rps[0:S, :])

                # rwt[v, (y, g, x)]: one matmul pair per column y
                rwt_all = qhpool.tile([S, S * 44], BF16, tag="rwt_all", name="rwta")
                for y in range(S):
                    wps = ps2.tile([S, 44], F32, tag="mm484", name=f"rwtp{y}")
                    cs = slice(y * S, (y + 1) * S)
                    nc.tensor.matmul(wps[0:S, :], rw1[:, cs], vg1[:, :, :, y],
                                     start=True, stop=False)
                    nc.tensor.matmul(wps[0:S, :], rw2[:, cs], vg2[:, :, :, y],
                                     start=False, stop=True)
                    nc.vector.tensor_copy(rwt_all[:, y * 44:(y + 1) * 44],
                                          wps[0:S, :])

                rhv = rht_all[:].rearrange("p (a g b) -> p a g b", g=HEAD, b=S)
                rwv = rwt_all[:].rearrange("p (b g a) -> p b g a", g=HEAD, a=SLABR)
                for h in range(HEAD):
                    rsb = relout_pool.tile([S, NPOS], BF16, tag=f"rhtsb{h}",
                                           name=f"rhtsb{h}")
                    nc.vector.tensor_copy(
                        rsb[:].rearrange("p (a b) -> p a b", b=S), rhv[:, :, h, :])
                    rht_sb[h] = rsb
                    wsb = relout_pool.tile([S, NPOS], BF16, tag=f"rwtsb{h}",
                                           name=f"rwtsb{h}")
                    nc.vector.tensor_copy(
                        wsb[:].rearrange("p (a b) -> p a b", b=S),
                        rwv[:, :, h, :].rearrange("p b a -> p a b"))
                    rwt_sb[h] = wsb
                nc.leave_named_scope("rel", sid, False)

            # ---- stage 5: attention, two heads interleaved per pass ----
            def build_k_tiles(h):
                ka1 = augpool.tile([128, HW], BF16, tag="ka1", name=f"ka1h{h}")
                ka2 = augpool.tile([128, HW], BF16, tag="ka2", name=f"ka2h{h}")
                ago = ag_out_a if h < 2 else ag_out_b
                hh = h % 2
                for sl in range(NSLAB):
                    cs = slice(sl * NPOS, (sl + 1) * NPOS)
                    base = sl * (CIN // 2) + hh * D
                    nc.sync.dma_start(ka1[:, cs], ago[base:base + 128, :])
                    nc.sync.dma_start(ka2[0:64, cs], ago[base + 128:base + D, :])
                nc.sync.dma_start(ka2[64:128, :], d_in["k2c"].ap())
                return ka1, ka2

            def build_q1(h):
                qa1 = augpool.tile([128, NPOS], BF16, tag="qa1", name=f"qa1h{h}")
                for (t_i, off, d0, take) in q_row_splits(h * D, 128):
                    nc.scalar.dma_start(qa1[d0:d0 + take, :],
                                        q_sb[t_i][off:off + take, :])
                return qa1

            def build_q2(h):
                qa2 = augpool.tile([128, NPOS], BF16, tag="qa2", name=f"qa2h{h}")
                qa3 = augpool.tile([24, NPOS], BF16, tag="qa3", name=f"qa3h{h}")
                for (t_i, off, d0, take) in q_row_splits(h * D + 128, 64):
                    nc.scalar.dma_start(qa2[d0:d0 + take, :],
                                        q_sb[t_i][off:off + take, :])
                nc.scalar.dma_start(qa2[64:108, :], rht_sb[h][:])
                nc.scalar.dma_start(qa2[108:128, :], rwt_sb[h][0:20, :])
                nc.scalar.dma_start(qa3[:], rwt_sb[h][20:44, :])
                return qa2, qa3

            def emit_av(av, item):
                h, idx, jc, ex = item
                jn = JCH
                av1, av2 = av[h]
                vt = vt_t[jc]
                c0 = h * (D + 1)
                nc.tensor.matmul(av1[:], vt[0:jn, c0:c0 + 128], ex[:jn],
                                 start=(idx == 0), stop=(idx == NJC - 1))
                nc.tensor.matmul(av2[:], vt[0:jn, c0 + 128:c0 + 193], ex[:jn],
                                 start=(idx == 0), stop=(idx == NJC - 1))

            def emit_proj_half(half, heads):
                sid_pj = nc.enter_named_scope(f"proj{half}", False)[0]
                for mb in range(KC_X):
                    pps = ps2.tile([128, NPOS], F32, tag="mm484",
                                   name=f"pj{half}_{mb}")
                    ops = []
                    for h in heads:
                        ops.append((wpa_t[h], at_a[h], 128))
                        ops.append((wpb_t[h], at_b[h], 64))
                    for i, (w, a, pr) in enumerate(ops):
                        nc.tensor.matmul(pps[:], w[0:pr, mb * 128:(mb + 1) * 128],
                                         a[0:pr, :],
                                         start=(i == 0), stop=(i == len(ops) - 1))
                    if half == 0:
                        # fold the residual in here, off the tail's critical path
                        nc.vector.tensor_add(pjpart_t[mb][:], pps[:], xres_t[mb][:])
                    else:
                        ot = work.tile([128, NPOS], F32, tag="outsb", bufs=1)
                        nc.vector.tensor_add(ot[:], pps[:], pjpart_t[mb][:])
                        nc.sync.dma_start(out_d.ap()[mb * 128:(mb + 1) * 128, :],
                                          ot[:])
                nc.leave_named_scope(f"proj{half}", sid_pj, False)

            # prefetch pair0's K tiles + q rows while rel computes
            pend_tiles = {}
            for h in (0, 1):
                ka1, ka2 = build_k_tiles(h)
                pend_tiles[h] = [ka1, ka2, build_q1(h)]
            emit_rel()
            sD.close()
            for h in (0, 1):
                qa2, qa3 = build_q2(h)
                pend_tiles[h] += [qa2, qa3]

            for hp in range(HEAD // 2):
                heads = (2 * hp, 2 * hp + 1)
                sid_h = nc.enter_named_scope(f"pair{hp}", False)[0]
                tiles = {h: pend_tiles.pop(h) for h in heads}
                av = {h: (psE.tile([128, NPOS], F32, tag=f"av1_{h % 2}", name=f"av1h{h}"),
                          psE.tile([65, NPOS], F32, tag=f"av2_{h % 2}", name=f"av2h{h}"))
                      for h in heads}
                pend = []   # (h, idx, jc, ex) exp tiles whose AV matmuls are deferred
                for idx, jc in enumerate(range(NJC)):
                    j0 = jc * JCH
                    jn = JCH
                    for h in heads:
                        ka1, ka2, qa1, qa2, qa3 = tiles[h]
                        sps = psS.tile([JCH, NPOS], F32, tag="simps")
                        nc.tensor.matmul(sps[:jn], ka1[:, j0:j0 + jn], qa1[:],
                                         start=True, stop=False)
                        nc.tensor.matmul(sps[:jn], ka2[:, j0:j0 + jn], qa2[:],
                                         start=False, stop=False)
                        nc.tensor.matmul(sps[:jn], ka3[:, j0:j0 + jn], qa3[:],
                                         start=False, stop=True)
                        ex = exppool.tile([JCH, NPOS], BF16, tag="expt")
                        nc.scalar.activation(ex[:jn], sps[:jn], AF.Exp,
                                             bias=shift_t[:jn, :])
                        pend.append((h, idx, jc, ex))
                        if len(pend) > 2:
                            emit_av(av, pend.pop(0))
                # prefetch pair1's tiles while pair0 finishes
                if hp == 0:
                    for h in (2, 3):
                        ka1, ka2 = build_k_tiles(h)
                        qa2, qa3 = build_q2(h)
                        pend_tiles[h] = [ka1, ka2, build_q1(h), qa2, qa3]
                while pend:
                    emit_av(av, pend.pop(0))

                for h in heads:
                    av1, av2 = av[h]
                    # normalize: denominator sits on partition 64 of av2
                    den = work.tile([65, NPOS], F32, tag="densb", bufs=1)
                    nc.vector.reciprocal(den[64:65, :], av2[64:65, :])
                    rrec = work.tile([1, NPOS], F32, tag="rrec", bufs=1)
                    nc.sync.dma_start(rrec[:], den[64:65, :])
                    rall = work.tile([128, NPOS], F32, tag="rall", bufs=1)
                    nc.gpsimd.partition_broadcast(rall[:], rrec[:])
                    nc.vector.tensor_tensor(at_a[h][:], av1[:], rall[:],
                                            mybir.AluOpType.mult)
                    nc.vector.tensor_tensor(at_b[h][:], av2[0:64, :], rall[0:64, :],
                                            mybir.AluOpType.mult)
                nc.leave_named_scope(f"pair{hp}", sid_h, False)
                # half of the proj contraction as soon as its heads are done
                emit_proj_half(hp, heads)
            sE.close()
            relout.close()
            sPJ.close()

    nc.compile()
    return nc


_NC_CACHE = None
last_exec_time_ns = None
last_results = None


def kernel(**inputs):
    global _NC_CACHE, last_exec_time_ns, last_results
    if _NC_CACHE is None:
        _NC_CACHE = build_nc()
    in_maps = make_in_maps(inputs)
    trace = bool(int(os.environ.get("AGG_TRACE", "0")))
    res = run_bass_kernel_spmd(_NC_CACHE, in_maps, list(range(NCORES)), trace=trace)
    last_exec_time_ns = res.exec_time_ns
    last_results = res
    final = np.empty((B, CIN, S, S), np.float32)
    for c in range(NCORES):
        b, s = c // 4, c % 4
        final[b, :, s * SLABR:(s + 1) * SLABR, :] = (
            res.results[c]["out"].reshape(CIN, SLABR, S))
    return final


# revision 22
# speedup vs baseline: 1.0175x; 1.0175x over previous
"""Trainium2 Bass kernel for nn_Aggregation_89575837925422.

Module: feats = [x, dconv3(x), dconv5(x), dconv7(x)] (1920 ch); qk = w_qk@feats;
4-head attention with relative-position biases; out = x + gamma*proj(attn@v).

Sharding across 8 NeuronCores: core c = (batch b=c//4, row-slab s=c%4, 11 rows).
Each core: all conv channels for its slab -> k(all heads)+q at slab positions ->
AllGather of k within the batch group -> v^T for its slab positions, AllGathered
across the group (each core computes 1/4 of v^T) -> all 4 heads' attention for
its query slab (augmented-K matmul folds rel-pos biases into the logit matmul;
softmax without max-subtraction via constant shift; denominators via a ones
column in the v operand) -> proj + residual. Output per core: (768, 484).

All matmul operands are bf16 (f32 PSUM accumulation): full PE rate, half the
HBM/SBUF traffic of f32r, keeping the tensor engine fed so its DVFS clock stays
in the high p-states. f32 is kept for the residual x, softmax denominators, and
the final output.
"""
import os
import sys

sys.path.insert(0, "/opt/trn_rl_repo")

from contextlib import ExitStack

import ml_dtypes
import numpy as np

import concourse.bacc as bacc
import concourse.mybir as mybir
import concourse.tile as tile
from concourse.bass_utils import run_bass_kernel_spmd

dt = mybir.dt
F32 = dt.float32
BF16 = dt.bfloat16
AF = mybir.ActivationFunctionType
BF = ml_dtypes.bfloat16

# ---- problem constants (hardcoded; kernel.py must be self-contained) ----
B = 2
CIN = 768
S = 44
HW = S * S              # 1936
HEAD = 4
D = 192                 # head dim
CO = 384                # conv out channels per dilation
DILS = (3, 5, 7)
NSLAB = 4
SLABR = 11              # rows per slab
NPOS = SLABR * S        # 484
MAXPOS = 100
KC_X = CIN // 128       # 6
C_CONV = 3 * CO         # 1152
MB_CONV = C_CONV // 128  # 9
KC_F = KC_X + MB_CONV   # 15
PADR = 7 + SLABR + 7    # 25
PADW = 64               # 7+44+13 zero pad, 128B row pitch in bf16
NJC = 16                # j-chunks of 121 rows each (16*121 = 1936)
JCH = HW // NJC         # 121
VT_W = HEAD * (D + 1)   # 772
EXP_SHIFT = -20.0       # exp(sim - 20); cancels in softmax, avoids fp32 overflow
NCORES = 8
GROUPS = [[0, 1, 2, 3], [4, 5, 6, 7]]


# ---------------------------------------------------------------------------
# host-side input preparation
# ---------------------------------------------------------------------------

def prep_shared(inputs):
    out = {}
    w_conv = np.empty((MB_CONV, 128, 54 * 128), np.float32)
    b_conv = np.empty((128, MB_CONV), np.float32)
    for mb in range(MB_CONV):
        dil_i, mloc = mb // 3, mb % 3
        Wd = np.asarray(inputs[f"w_d{DILS[dil_i]}"], np.float32)
        Wb = Wd[mloc * 128:(mloc + 1) * 128]              # (128m, 768ci, 3, 3)
        t = Wb.reshape(128, KC_X, 128, 3, 3).transpose(3, 4, 1, 2, 0)
        w_conv[mb] = t.reshape(54, 128, 128).transpose(1, 0, 2).reshape(128, 54 * 128)
        b_conv[:, mb] = np.asarray(inputs[f"b_d{DILS[dil_i]}"], np.float32)[mloc * 128:(mloc + 1) * 128]
    out["w_conv"] = w_conv.astype(BF)
    out["b_conv"] = b_conv

    w_qk = np.asarray(inputs["w_qk"], np.float32)         # (1536, 1920)
    qscale = HEAD ** -0.5
    w_qk_l = np.empty((12, 128, KC_F * 128), np.float32)
    for blk in range(12):
        rows = w_qk[blk * 128:(blk + 1) * 128]
        scale = qscale if blk < 6 else 1.0
        w_qk_l[blk] = (rows * scale).reshape(128, KC_F, 128).transpose(2, 1, 0).reshape(
            128, KC_F * 128)
    out["w_qk"] = w_qk_l.astype(BF)

    w_v = np.asarray(inputs["w_v"], np.float32)           # (768, 768) [o, c]
    w_vt = np.zeros((KC_X, 128, VT_W), np.float32)
    for kc in range(KC_X):
        blockT = w_v[:, kc * 128:(kc + 1) * 128].T
        for h in range(HEAD):
            w_vt[kc][:, h * (D + 1):h * (D + 1) + D] = blockT[:, h * D:(h + 1) * D]
    out["w_vt"] = w_vt.astype(BF)

    gamma = float(np.asarray(inputs["gamma"]).reshape(-1)[0])
    w_proj = np.asarray(inputs["w_proj"], np.float32)
    wpa = np.empty((HEAD, 128, CIN), np.float32)
    wpb = np.empty((HEAD, 64, CIN), np.float32)
    for h in range(HEAD):
        wpa[h] = gamma * w_proj[:, h * D:h * D + 128].T
        wpb[h] = gamma * w_proj[:, h * D + 128:(h + 1) * D].T
    out["w_proj_a"] = wpa.astype(BF)
    out["w_proj_b"] = wpb.astype(BF)

    rel_w = np.asarray(inputs["rel_w"], np.float32)
    iy = np.arange(S)
    rw = rel_w[iy[None, :] - iy[:, None] + MAXPOS - 1]    # (y, v, d)
    out["rw"] = np.ascontiguousarray(rw.transpose(2, 0, 1).reshape(D, S * S)).astype(BF)

    j = np.arange(HW)
    U = (j[None, :] // S == np.arange(S)[:, None]).astype(np.float32)
    V = (j[None, :] % S == np.arange(S)[:, None]).astype(np.float32)
    out["k2c"] = np.ascontiguousarray(np.concatenate([U, V[:20]], axis=0)).astype(BF)  # (64, 1936)
    out["k3c"] = np.ascontiguousarray(V[20:44]).astype(BF)                             # (24, 1936)
    return out


def prep_core(inputs, core):
    b, s = core // 4, core % 4
    r0 = s * SLABR
    x = np.asarray(inputs["x"], np.float32)[b]
    out = {}
    xp = np.zeros((KC_X, 128, PADR, PADW), np.float32)
    rlo, rhi = r0 - 7, r0 + SLABR + 7
    glo, ghi = max(rlo, 0), min(rhi, S)
    xr = x.reshape(KC_X, 128, S, S)
    xp[:, :, glo - rlo:ghi - rlo, 7:7 + S] = xr[:, :, glo:ghi, :]
    out["x_pad"] = xp.reshape(KC_X, 128, PADR * PADW).astype(BF)
    xf = xr.reshape(KC_X, 128, HW)
    out["x_full"] = np.ascontiguousarray(xf).astype(BF)
    xs = np.ascontiguousarray(xf[:, :, r0 * S:r0 * S + NPOS])
    out["x_slab_bf"] = xs.astype(BF)
    out["x_res"] = xs
    rel_h = np.asarray(inputs["rel_h"], np.float32)
    ix = np.arange(S)
    rh = rel_h[ix[None, :] - ix[:, None] + MAXPOS - 1]    # (x, u, d)
    out["rh"] = np.ascontiguousarray(
        rh[r0:r0 + SLABR].transpose(2, 0, 1).reshape(D, NPOS)).astype(BF)
    return out


def make_in_maps(inputs):
    shared = prep_shared(inputs)
    in_maps = []
    for c in range(NCORES):
        m = dict(shared)
        m.update(prep_core(inputs, c))
        in_maps.append(m)
    return in_maps


# ---------------------------------------------------------------------------
# Bass program (SPMD, identical on all 8 cores)
# ---------------------------------------------------------------------------

def build_nc():
    nc = bacc.Bacc("TRN2", target_bir_lowering=False, debug=False,
                   num_devices=NCORES)

    d_in = {}
    for name, shape in [
        ("x_pad", [KC_X, 128, PADR * PADW]),
        ("x_slab_bf", [KC_X, 128, NPOS]),
        ("x_full", [KC_X, 128, HW]),
        ("w_conv", [MB_CONV, 128, 54 * 128]),
        ("w_qk", [12, 128, KC_F * 128]),
        ("w_vt", [KC_X, 128, VT_W]),
        ("w_proj_a", [HEAD, 128, CIN]),
        ("w_proj_b", [HEAD, 64, CIN]),
        ("rh", [D, NPOS]),
        ("rw", [D, HW]),
        ("k2c", [64, HW]),
        ("k3c", [24, HW]),
    ]:
        d_in[name] = nc.dram_tensor(name, shape, BF16, kind="ExternalInput")
    d_in["b_conv"] = nc.dram_tensor("b_conv", [128, MB_CONV], F32, kind="ExternalInput")
    d_in["x_res"] = nc.dram_tensor("x_res", [KC_X, 128, NPOS], F32, kind="ExternalInput")
    out_d = nc.dram_tensor("out", [CIN, NPOS], F32, kind="ExternalOutput")

    with tile.TileContext(nc) as tc:
        with ExitStack() as ctx:
            # persistent pools (whole kernel)
            consts = ctx.enter_context(tc.tile_pool(name="consts", bufs=1))
            work = ctx.enter_context(tc.tile_pool(name="work", bufs=2))
            vtpool = ctx.enter_context(tc.tile_pool(name="vtpool", bufs=1))
            qpool = ctx.enter_context(tc.tile_pool(name="qpool", bufs=1))
            ps2 = ctx.enter_context(tc.tile_pool(name="ps2", bufs=2, space="PSUM"))
            dram = ctx.enter_context(tc.tile_pool(name="dram", bufs=1, space="DRAM"))

            # phase-scoped pools (closed manually, strict LIFO order)
            sXF = ExitStack()  # x_full for the local v^T compute
            xfpool = sXF.enter_context(tc.tile_pool(name="xfpool", bufs=1))
            sB = ExitStack()   # conv outputs + qk weights (closes after qk)
            convout = sB.enter_context(tc.tile_pool(name="convout", bufs=1))
            qkw = sB.enter_context(tc.tile_pool(name="qkw", bufs=2))
            sA = ExitStack()   # conv inputs/weights (closes after conv)
            xpool = sA.enter_context(tc.tile_pool(name="xpool", bufs=1))
            convw = sA.enter_context(tc.tile_pool(name="convw", bufs=8))

            # ---- persistent input tiles ----
            # x_pad first on the sync queue: the first conv matmul needs it
            xpad_t = []
            for kc in range(KC_X):
                t = xpool.tile([128, PADR * PADW], BF16, tag=f"xpad{kc}")
                nc.sync.dma_start(t[:], d_in["x_pad"].ap()[kc])
                xpad_t.append(t)
            xslab_t = []
            for kc in range(KC_X):
                t = consts.tile([128, NPOS], BF16, tag=f"xslab{kc}")
                nc.scalar.dma_start(t[:], d_in["x_slab_bf"].ap()[kc])
                xslab_t.append(t)
            bconv_t = consts.tile([128, MB_CONV], F32, tag="bconv")
            nc.scalar.dma_start(bconv_t[:], d_in["b_conv"].ap())
            # rel-pos tables: early, on the scalar queue (gpsimd's queue stalls
            # behind collective triggers)
            rh1 = consts.tile([128, NPOS], BF16, tag="rh1")
            nc.scalar.dma_start(rh1[:], d_in["rh"].ap()[0:128, :])
            rh2 = consts.tile([64, NPOS], BF16, tag="rh2")
            nc.scalar.dma_start(rh2[:], d_in["rh"].ap()[128:D, :])
            rw1 = consts.tile([128, HW], BF16, tag="rw1")
            nc.scalar.dma_start(rw1[:], d_in["rw"].ap()[0:128, :])
            rw2 = consts.tile([64, HW], BF16, tag="rw2")
            nc.scalar.dma_start(rw2[:], d_in["rw"].ap()[128:D, :])
            wvt_t = []
            for kc in range(KC_X):
                t = consts.tile([128, VT_W], BF16, tag=f"wvt{kc}")
                nc.gpsimd.dma_start(t[:], d_in["w_vt"].ap()[kc])
                wvt_t.append(t)

            # ---- DRAM scratch ----
            ag_in_a = dram.tile([CIN // 2, NPOS], BF16, tag="ag_in_a")
            ag_in_b = dram.tile([CIN // 2, NPOS], BF16, tag="ag_in_b")
            ag_out_a = dram.tile([NSLAB * CIN // 2, NPOS], BF16, tag="ag_out_a")
            ag_out_b = dram.tile([NSLAB * CIN // 2, NPOS], BF16, tag="ag_out_b")

            # ---- stage 1: dilated convs for this slab ----
            sid_conv = nc.enter_named_scope("conv", False)[0]
            conv_out = []
            for mb in range(MB_CONV):
                cps = ps2.tile([128, NPOS], F32, tag="mm484")
                first = True
                for kh in range(3):
                    for kw in range(3):
                        t_idx = kh * 3 + kw
                        wt = convw.tile([128, KC_X * 128], BF16, tag="convw")
                        nc.sync.dma_start(
                            wt[:],
                            d_in["w_conv"].ap()[mb][:, t_idx * KC_X * 128:(t_idx + 1) * KC_X * 128],
                        )
                        dil = DILS[mb // 3]
                        dr, dw = dil * (kh - 1), dil * (kw - 1)
                        for kc in range(KC_X):
                            rhs = xpad_t[kc][:].rearrange(
                                "p (r w) -> p r w", w=PADW
                            )[:, 7 + dr:7 + dr + SLABR, 7 + dw:7 + dw + S]
                            last = (t_idx == 8 and kc == KC_X - 1)
                            nc.tensor.matmul(
                                cps[:], wt[:, kc * 128:(kc + 1) * 128], rhs,
                                start=first, stop=last,
                            )
                            first = False
                co = convout.tile([128, NPOS], BF16, tag=f"conv{mb}")
                nc.scalar.activation(co[:], cps[:], AF.Identity,
                                     bias=bconv_t[:, mb:mb + 1])
                conv_out.append(co)
            sA.close()
            nc.leave_named_scope("conv", sid_conv, False)

            def feats_rhs(kc):
                if kc < KC_X:
                    return xslab_t[kc][:]
                return conv_out[kc - KC_X][:]

            # q lives in SBUF for the whole kernel: 6 blocks of 128 rows
            q_sb = [qpool.tile([128, NPOS], BF16, tag=f"qsb{b}", name=f"qsb{b}")
                    for b in range(KC_X)]

            def q_row_splits(r0g, n):
                """Split q global rows [r0g, r0g+n) into (tile, src_off, dst_off, n)."""
                parts = []
                done = 0
                while done < n:
                    t_i, off = divmod(r0g + done, 128)
                    take = min(128 - off, n - done)
                    parts.append((t_i, off, done, take))
                    done += take
                return parts

            # ---- stage 2a: k blocks + AllGathers ----
            sid_qk = nc.enter_named_scope("qk_k", False)[0]

            def qk_block(blk, dst, dst_row):
                qps = ps2.tile([128, NPOS], F32, tag="mm484", name=f"qps{blk}")
                wt = qkw.tile([128, KC_F * 128], BF16, tag="qkw", name=f"qkw{blk}")
                nc.scalar.dma_start(wt[:], d_in["w_qk"].ap()[blk])
                for kc in range(KC_F):
                    nc.tensor.matmul(qps[:], wt[:, kc * 128:(kc + 1) * 128],
                                     feats_rhs(kc),
                                     start=(kc == 0), stop=(kc == KC_F - 1))
                if dst is None:
                    nc.vector.tensor_copy(q_sb[blk][:], qps[:])
                else:
                    sb = qkw.tile([128, NPOS], BF16, tag="qkout", name=f"qko{blk}")
                    nc.vector.tensor_copy(sb[:], qps[:])
                    nc.sync.dma_start(dst[dst_row:dst_row + 128, :], sb[:])

            for blk in range(6, 9):
                qk_block(blk, ag_in_a, (blk - 6) * 128)
            sid_ag = nc.enter_named_scope("allgather", False)[0]
            nc.gpsimd.collective_compute(
                "AllGather", mybir.AluOpType.bypass,
                ins=[ag_in_a[:]], outs=[ag_out_a[:]], replica_groups=GROUPS,
            )
            nc.leave_named_scope("allgather", sid_ag, False)
            for blk in range(9, 12):
                qk_block(blk, ag_in_b, (blk - 9) * 128)
            sid_ag2 = nc.enter_named_scope("allgather2", False)[0]
            nc.gpsimd.collective_compute(
                "AllGather", mybir.AluOpType.bypass,
                ins=[ag_in_b[:]], outs=[ag_out_b[:]], replica_groups=GROUPS,
            )
            nc.leave_named_scope("allgather2", sid_ag2, False)
            nc.leave_named_scope("qk_k", sid_qk, False)

            # x_full for the v^T stage: loaded now (gpsimd queue is free after
            # the gather triggers; loading at t=0 oversubscribes HBM and
            # starves the conv weight stream)
            xfull_t = []
            for kc in range(KC_X):
                t = xfpool.tile([128, HW], BF16, tag=f"xfull{kc}", name=f"xfull{kc}")
                nc.gpsimd.dma_start(t[:], d_in["x_full"].ap()[kc])
                xfull_t.append(t)

            # ---- stage 2b: q blocks (into SBUF) ----
            sid_qq = nc.enter_named_scope("qk_q", False)[0]
            for blk in range(6):
                qk_block(blk, None, 0)
            nc.leave_named_scope("qk_q", sid_qq, False)
            sB.close()

            # ---- stage 3: vT_wide = x^T @ w_vT (all positions, local) ----
            # sits between the gather triggers and the attention pairs so the
            # tensor engine stays busy while the collectives fly
            sid_vt = nc.enter_named_scope("vt", False)[0]
            sC = ExitStack()
            psC = sC.enter_context(tc.tile_pool(name="psC", bufs=2, space="PSUM"))
            vt_t = []
            for jc in range(NJC):
                j0 = jc * JCH
                vps = psC.tile([128, VT_W], F32, tag="vtps")
                for kc in range(KC_X):
                    lhsT = xfull_t[kc][:, j0:j0 + JCH]
                    nc.tensor.matmul(vps[:JCH, 0:512], lhsT, wvt_t[kc][:, 0:512],
                                     start=(kc == 0), stop=(kc == KC_X - 1))
                    nc.tensor.matmul(vps[:JCH, 512:VT_W], lhsT,
                                     wvt_t[kc][:, 512:VT_W],
                                     start=(kc == 0), stop=(kc == KC_X - 1))
                t = vtpool.tile([128, VT_W], BF16, tag=f"vt{jc}", name=f"vt{jc}")
                nc.vector.tensor_copy(t[:JCH], vps[:JCH])
                for h in range(HEAD):
                    col = h * (D + 1) + D
                    nc.vector.memset(t[:JCH, col:col + 1], 1.0)
                vt_t.append(t)
            sC.close()
            nc.leave_named_scope("vt", sid_vt, False)
            sXF.close()

            # proj weights + attn output tiles (persist to the end)
            sPJ = ExitStack()
            projpool = sPJ.enter_context(tc.tile_pool(name="projpool", bufs=1))
            wpa_t, wpb_t = [], []
            for h in range(HEAD):
                ta = projpool.tile([128, CIN], BF16, tag=f"wpa{h}", name=f"wpa{h}")
                nc.sync.dma_start(ta[:], d_in["w_proj_a"].ap()[h])
                wpa_t.append(ta)
                tb = projpool.tile([64, CIN], BF16, tag=f"wpb{h}", name=f"wpb{h}")
                nc.sync.dma_start(tb[:], d_in["w_proj_b"].ap()[h])
                wpb_t.append(tb)
            # attention outputs, written in place by the normalize multiply
            at_a = [projpool.tile([128, NPOS], BF16, tag=f"ata{h}", name=f"ata{h}")
                    for h in range(HEAD)]
            at_b = [projpool.tile([64, NPOS], BF16, tag=f"atb{h}", name=f"atb{h}")
                    for h in range(HEAD)]
            # residual x (f32) for the final add; loaded late on gpsimd queue
            xres_t = []
            for kc in range(KC_X):
                t = projpool.tile([128, NPOS], F32, tag=f"xres{kc}", name=f"xres{kc}")
                nc.gpsimd.dma_start(t[:], d_in["x_res"].ap()[kc])
                xres_t.append(t)
            # f32 partial proj accumulators (pair0's half of the contraction)
            pjpart_t = [projpool.tile([128, NPOS], F32, tag=f"pjp{mb}", name=f"pjp{mb}")
                        for mb in range(KC_X)]

            # rel-pos tables + attention staging pools
            relout = ExitStack()
            relout_pool = relout.enter_context(tc.tile_pool(name="relout", bufs=1))
            sE = ExitStack()
            augpool = sE.enter_context(tc.tile_pool(name="augpool", bufs=4))
            aug1pool = sE.enter_context(tc.tile_pool(name="aug1pool", bufs=1))
            exppool = sE.enter_context(tc.tile_pool(name="exppool", bufs=5))
            psE = sE.enter_context(tc.tile_pool(name="psE", bufs=1, space="PSUM"))
            psS = sE.enter_context(tc.tile_pool(name="psS", bufs=2, space="PSUM"))
            shift_t = consts.tile([128, 1], F32, tag="shift")
            nc.vector.memset(shift_t[:], EXP_SHIFT)
            ka3 = aug1pool.tile([24, HW], BF16, tag="ka3")
            nc.sync.dma_start(ka3[:], d_in["k3c"].ap())

            sD = ExitStack()
            qhpool = sD.enter_context(tc.tile_pool(name="qhpool", bufs=1))

            rht_sb = {}
            rwt_sb = {}

            def emit_rel():
                sid = nc.enter_named_scope("rel", False)[0]
                # q gathered once, x-major layout: col = a*176 + g*44 + b
                qhx1 = qhpool.tile([128, SLABR * 176], BF16, tag="qhx1", name="qhx1")
                qhx2 = qhpool.tile([64, SLABR * 176], BF16, tag="qhx2", name="qhx2")
                vx1 = qhx1[:].rearrange("p (a g b) -> p a g b", g=HEAD, b=S)
                vx2 = qhx2[:].rearrange("p (a g b) -> p a g b", g=HEAD, b=S)
                # same data viewed (g, a) per fixed y for the rwt matmuls
                vg1 = qhx1[:].rearrange("p (a g b) -> p g a b", g=HEAD, b=S)
                vg2 = qhx2[:].rearrange("p (a g b) -> p g a b", g=HEAD, b=S)
                for g in range(HEAD):
                    for (t_i, off, d0, take) in q_row_splits(g * D, 128):
                        src = q_sb[t_i][off:off + take, :]
                        nc.scalar.dma_start(vx1[d0:d0 + take, :, g, :], src)
                    for (t_i, off, d0, take) in q_row_splits(g * D + 128, 64):
                        src = q_sb[t_i][off:off + take, :]
                        nc.scalar.dma_start(vx2[d0:d0 + take, :, g, :], src)

                # rht[u, (x, g, y)]: one matmul pair per slab row x
                rht_all = qhpool.tile([S, SLABR * 176], BF16, tag="rht_all",
                                       name="rhta")
                for xl in range(SLABR):
                    rps = ps2.tile([S, 176], F32, tag="mm484", name=f"rhtp{xl}")
                    cs = slice(xl * S, (xl + 1) * S)
                    nc.tensor.matmul(rps[0:S, :], rh1[:, cs],
                                     qhx1[:, xl * 176:(xl + 1) * 176],
                                     start=True, stop=False)
                    nc.tensor.matmul(rps[0:S, :], rh2[:, cs],
                                     qhx2[:, xl * 176:(xl + 1) * 176],
                                     start=False, stop=True)
                    nc.vector.tensor_copy(rht_all[:, xl * 176:(xl + 1) * 176],
                                          rps[0:S, :])

                # rwt[v, (y, g, x)]: one matmul pair per column y
                rwt_all = qhpool.tile([S, S * 44], BF16, tag="rwt_all", name="rwta")
                for y in range(S):
                    wps = ps2.tile([S, 44], F32, tag="mm484", name=f"rwtp{y}")
                    cs = slice(y * S, (y + 1) * S)
                    nc.tensor.matmul(wps[0:S, :], rw1[:, cs], vg1[:, :, :, y],
                                     start=True, stop=False)
                    nc.tensor.matmul(wps[0:S, :], rw2[:, cs], vg2[:, :, :, y],
                                     start=False, stop=True)
                    nc.vector.tensor_copy(rwt_all[:, y * 44:(y + 1) * 44],
                                          wps[0:S, :])

                rhv = rht_all[:].rearrange("p (a g b) -> p a g b", g=HEAD, b=S)
                rwv = rwt_all[:].rearrange("p (b g a) -> p b g a", g=HEAD, a=SLABR)
                for h in range(HEAD):
                    rsb = relout_pool.tile([S, NPOS], BF16, tag=f"rhtsb{h}",
                                           name=f"rhtsb{h}")
                    nc.vector.tensor_copy(
                        rsb[:].rearrange("p (a b) -> p a b", b=S), rhv[:, :, h, :])
                    rht_sb[h] = rsb
                    wsb = relout_pool.tile([S, NPOS], BF16, tag=f"rwtsb{h}",
                                           name=f"rwtsb{h}")
                    nc.vector.tensor_copy(
                        wsb[:].rearrange("p (a b) -> p a b", b=S),
                        rwv[:, :, h, :].rearrange("p b a -> p a b"))
                    rwt_sb[h] = wsb
                nc.leave_named_scope("rel", sid, False)

            # ---- stage 5: attention, two heads interleaved per pass ----
            def build_k_tiles(h):
                ka1 = augpool.tile([128, HW], BF16, tag="ka1", name=f"ka1h{h}")
                ka2 = augpool.tile([128, HW], BF16, tag="ka2", name=f"ka2h{h}")
                ago = ag_out_a if h < 2 else ag_out_b
                hh = h % 2
                for sl in range(NSLAB):
                    cs = slice(sl * NPOS, (sl + 1) * NPOS)
                    base = sl * (CIN // 2) + hh * D
                    nc.sync.dma_start(ka1[:, cs], ago[base:base + 128, :])
                    nc.sync.dma_start(ka2[0:64, cs], ago[base + 128:base + D, :])
                nc.sync.dma_start(ka2[64:128, :], d_in["k2c"].ap())
                return ka1, ka2

            def build_q1(h):
                qa1 = augpool.tile([128, NPOS], BF16, tag="qa1", name=f"qa1h{h}")
                for (t_i, off, d0, take) in q_row_splits(h * D, 128):
                    nc.scalar.dma_start(qa1[d0:d0 + take, :],
                                        q_sb[t_i][off:off + take, :])
                return qa1

            def build_q2(h):
                qa2 = augpool.tile([128, NPOS], BF16, tag="qa2", name=f"qa2h{h}")
                qa3 = augpool.tile([24, NPOS], BF16, tag="qa3", name=f"qa3h{h}")
                for (t_i, off, d0, take) in q_row_splits(h * D + 128, 64):
                    nc.scalar.dma_start(qa2[d0:d0 + take, :],
                                        q_sb[t_i][off:off + take, :])
                nc.scalar.dma_start(qa2[64:108, :], rht_sb[h][:])
                nc.scalar.dma_start(qa2[108:128, :], rwt_sb[h][0:20, :])
                nc.scalar.dma_start(qa3[:], rwt_sb[h][20:44, :])
                return qa2, qa3

            def emit_av(av, item):
                h, idx, jc, ex = item
                jn = JCH
                av1, av2 = av[h]
                vt = vt_t[jc]
                c0 = h * (D + 1)
                nc.tensor.matmul(av1[:], vt[0:jn, c0:c0 + 128], ex[:jn],
                                 start=(idx == 0), stop=(idx == NJC - 1))
                nc.tensor.matmul(av2[:], vt[0:jn, c0 + 128:c0 + 193], ex[:jn],
                                 start=(idx == 0), stop=(idx == NJC - 1))

            def emit_proj_half(half, heads):
                sid_pj = nc.enter_named_scope(f"proj{half}", False)[0]
                for mb in range(KC_X):
                    pps = ps2.tile([128, NPOS], F32, tag="mm484",
                                   name=f"pj{half}_{mb}")
                    ops = []
                    for h in heads:
                        ops.append((wpa_t[h], at_a[h], 128))
                        ops.append((wpb_t[h], at_b[h], 64))
                    for i, (w, a, pr) in enumerate(ops):
                        nc.tensor.matmul(pps[:], w[0:pr, mb * 128:(mb + 1) * 128],
                                         a[0:pr, :],
                                         start=(i == 0), stop=(i == len(ops) - 1))
                    if half == 0:
                        # fold the residual in here, off the tail's critical path
                        nc.vector.tensor_add(pjpart_t[mb][:], pps[:], xres_t[mb][:])
                    else:
                        ot = work.tile([128, NPOS], F32, tag="outsb", bufs=1)
                        nc.vector.tensor_add(ot[:], pps[:], pjpart_t[mb][:])
                        nc.sync.dma_start(out_d.ap()[mb * 128:(mb + 1) * 128, :],
                                          ot[:])
                nc.leave_named_scope(f"proj{half}", sid_pj, False)

            # prefetch pair0's K tiles + q rows while rel computes
            pend_tiles = {}
            for h in (0, 1):
                ka1, ka2 = build_k_tiles(h)
                pend_tiles[h] = [ka1, ka2, build_q1(h)]
            emit_rel()
            sD.close()
            for h in (0, 1):
                qa2, qa3 = build_q2(h)
                pend_tiles[h] += [qa2, qa3]

            for hp in range(HEAD // 2):
                heads = (2 * hp, 2 * hp + 1)
                sid_h = nc.enter_named_scope(f"pair{hp}", False)[0]
                tiles = {h: pend_tiles.pop(h) for h in heads}
                av = {h: (psE.tile([128, NPOS], F32, tag=f"av1_{h % 2}", name=f"av1h{h}"),
                          psE.tile([65, NPOS], F32, tag=f"av2_{h % 2}", name=f"av2h{h}"))
                      for h in heads}
                pend = []   # (h, idx, jc, ex) exp tiles whose AV matmuls are deferred
                for idx, jc in enumerate(range(NJC)):
                    j0 = jc * JCH
                    jn = JCH
                    for h in heads:
                        ka1, ka2, qa1, qa2, qa3 = tiles[h]
                        sps = psS.tile([JCH, NPOS], F32, tag="simps")
                        nc.tensor.matmul(sps[:jn], ka1[:, j0:j0 + jn], qa1[:],
                                         start=True, stop=False)
                        nc.tensor.matmul(sps[:jn], ka2[:, j0:j0 + jn], qa2[:],
                                         start=False, stop=False)
                        nc.tensor.matmul(sps[:jn], ka3[:, j0:j0 + jn], qa3[:],
                                         start=False, stop=True)
                        ex = exppool.tile([JCH, NPOS], BF16, tag="expt")
                        nc.scalar.activation(ex[:jn], sps[:jn], AF.Exp,
                                             bias=shift_t[:jn, :])
                        pend.append((h, idx, jc, ex))
                        if len(pend) > 2:
                            emit_av(av, pend.pop(0))
                # prefetch pair1's tiles while pair0 finishes
                if hp == 0:
                    for h in (2, 3):
                        ka1, ka2 = build_k_tiles(h)
                        qa2, qa3 = build_q2(h)
                        pend_tiles[h] = [ka1, ka2, build_q1(h), qa2, qa3]
                while pend:
                    emit_av(av, pend.pop(0))

                for h in heads:
                    av1, av2 = av[h]
                    # normalize: denominator sits on partition 64 of av2
                    den = work.tile([65, NPOS], F32, tag="densb", bufs=1)
                    nc.vector.reciprocal(den[64:65, :], av2[64:65, :])
                    rrec = work.tile([1, NPOS], F32, tag="rrec", bufs=1)
                    nc.sync.dma_start(rrec[:], den[64:65, :])
                    rall = work.tile([128, NPOS], F32, tag="rall", bufs=1)
                    nc.gpsimd.partition_broadcast(rall[:], rrec[:])
                    nc.vector.tensor_tensor(at_a[h][:], av1[:], rall[:],
                                            mybir.AluOpType.mult)
                    nc.vector.tensor_tensor(at_b[h][:], av2[0:64, :], rall[0:64, :],
                                            mybir.AluOpType.mult)
                nc.leave_named_scope(f"pair{hp}", sid_h, False)
                # half of the proj contraction as soon as its heads are done
                emit_proj_half(hp, heads)
            sE.close()
            relout.close()
            sPJ.close()

    nc.compile()
    return nc


_NC_CACHE = None
last_exec_time_ns = None
last_results = None


def kernel(**inputs):
    global _NC_CACHE, last_exec_time_ns, last_results
    if _NC_CACHE is None:
        _NC_CACHE = build_nc()
    in_maps = make_in_maps(inputs)
    trace = bool(int(os.environ.get("AGG_TRACE", "0")))
    res = run_bass_kernel_spmd(_NC_CACHE, in_maps, list(range(NCORES)), trace=trace)
    last_exec_time_ns = res.exec_time_ns
    last_results = res
    final = np.empty((B, CIN, S, S), np.float32)
    for c in range(NCORES):
        b, s = c // 4, c % 4
        final[b, :, s * SLABR:(s + 1) * SLABR, :] = (
            res.results[c]["out"].reshape(CIN, SLABR, S))
    return final
